# revision 12
# baseline (speedup 1.0000x reference)
"""Temporal GCN (segment-sum message passing) + LSTM on 8 Trainium2
NeuronCores.

Contract: kernel(**inputs) takes the FULL unsharded inputs (same keys as
setup_inputs()) and returns the FULL [T, N, H] float32 output.

Strategy (hardcoded for T=12, N=20000, E=640000, F=128, H=64, 8 cores):
  - Nodes sharded 8 ways (2500/core, padded to 2560). Host-side prep is
    index routing only: edges bucketed to the core owning dst, degree
    counts, per-(t, core) "slab" gather index lists (nodes ranked by
    local degree so slab j = j-th edge of every rank with degree > j,
    zero-padded to a fixed capacity profile), plus weight transposes.
  - Device schedule: a Stage-A pass first computes h' = dinv*(x @ W_gcn)
    for ALL timesteps (x arrives host-transposed feature-major fp16, so
    each 128-node block is one ldweights+matmul, no PE transposes),
    written to per-t DRAM tables. Then the per-t pipeline runs with the
    Pool engine (SWDGE) issuing slab dma_gathers back-to-back across all
    t (round-robin over 4 SWDGE queues); DVE slab accumulation, scale/
    bias/relu, dma_scatter_add unpermute, PE-transpose to feature-major
    and the LSTM step all overlap under the gather stream, with stages
    C/D of timestep t-1 emitted between t's gather windows.
  - x is replicated to all cores (each computes the full h' table);
    weights replicated; output written feature-major fp16 and assembled/
    transposed on host.
"""
import math
import os
import sys

# The kernel needs the axon/neuron jax platform; undo a CPU pin inherited
# from a caller that ran the jax reference first (must happen before jax
# is first imported in this process).
if os.environ.get("JAX_PLATFORMS") == "cpu" and "jax" not in sys.modules:
    del os.environ["JAX_PLATFORMS"]

sys.path.insert(0, "/opt/trn_rl_repo")

import numpy as np

import concourse.bass as bass
import concourse.bacc as bacc
import concourse.mybir as mybir
import concourse.tile as tile
from concourse.masks import make_identity
from concourse.library_config import mlp as mlp_lib
from concourse.bass_utils import run_bass_kernel_spmd

FP32 = mybir.dt.float32
FP16 = mybir.dt.float16
I16 = mybir.dt.int16
AF = mybir.ActivationFunctionType
OP = mybir.AluOpType

# ---- problem constants (hardcoded per contract)
T, N, E, F, H = 12, 20000, 640000, 128, 64
NCORES = 8
NLOC = N // NCORES            # 2500
NP = (NLOC + 127) // 128 * 128  # 2560
SL = NP // 128                # 20
G = NCORES * NP               # 20480
G4 = 4 * H
WSLOTS = 64                   # gather window: 64*128 = 8192 indices
LSTM_CHUNK = 512
ACHUNK = 8                    # Stage-A node blocks per PSUM bank
NQ = 4                        # SWDGE queues for gather round-robin


def _default_cbar():
    """Slab capacity profile (multiples of 128), derived from the max
    realized c_j = #{nodes: local degree > j} over (t, core) for the
    deterministic problem instance, plus margin."""
    pmax = [2500] * 18 + [2499, 2495, 2492, 2483, 2452, 2427, 2383, 2319,
                          2240, 2133, 1998, 1853, 1707, 1536, 1354, 1183,
                          1011, 866, 719, 579, 456, 365, 275, 208, 155,
                          111, 83, 57, 43, 33, 24, 15, 11, 9, 6, 5, 4,
                          3, 2, 2, 1, 1, 1, 1, 1, 1, 1]
    pmax = np.array(pmax + [1, 1], dtype=np.float64)
    marg = pmax + 4 + 2 * np.sqrt(pmax)
    cb = np.minimum(NP, np.ceil(marg / 128).astype(int) * 128)
    cb[0] = NP
    return tuple(int(v) for v in cb)


CBAR = _default_cbar()
SLOTS = [c // 128 for c in CBAR]
K = sum(SLOTS)
NW = -(-K // WSLOTS)
KPAD = NW * WSLOTS
ZROW = G


# ------------------------------------------------------------- host prep

def _host_prep(x, edge_index, W_gcn, b_gcn, W_ih, W_hh, b_ih, b_hh):
    x = np.asarray(x, dtype=np.float32)
    edge_index = np.asarray(edge_index)
    cbar = np.array(CBAR)
    obase = np.concatenate([[0], np.cumsum(cbar)])

    idxs = np.zeros((NCORES, T, NW, 128, WSLOTS * 8), dtype=np.int16)
    deg_node = np.ones((T, 128, G // 128), dtype=np.float32)
    deg_rank = np.ones((NCORES, T, 128, SL), dtype=np.float32)
    rank_node = np.zeros((NCORES, T, 128, NP // 16), dtype=np.int16)

    for t in range(T):
        src_t = edge_index[t, 0].astype(np.int64)
        dst_t = edge_index[t, 1].astype(np.int64)
        deg = np.bincount(dst_t, minlength=N) + 1
        order_e = np.argsort(dst_t, kind="stable")
        src_sorted = src_t[order_e]
        counts = np.bincount(dst_t, minlength=N)
        starts = np.concatenate([[0], np.cumsum(counts)])
        srow_sorted = (src_sorted // NLOC) * NP + (src_sorted % NLOC)
        for c in range(NCORES):
            lo, hi = c * NLOC, (c + 1) * NLOC
            dloc = deg[lo:hi]
            order = np.argsort(-dloc, kind="stable")
            dmax = int(dloc.max())
            if dmax > len(cbar):
                raise RuntimeError("slab overflow (depth)")
            c_j = np.array([(dloc > j).sum() for j in range(dmax)])
            if np.any(c_j > cbar[:dmax]):
                raise RuntimeError("slab overflow (width)")
            A = np.full((NLOC, dmax), ZROW, dtype=np.int64)
            cnt_loc = counts[lo:hi]
            nidx = np.repeat(np.arange(NLOC), cnt_loc)
            jj = np.arange(starts[lo], starts[hi]) - np.repeat(
                starts[lo:hi], cnt_loc)
            A[nidx, jj] = srow_sorted[starts[lo]:starts[hi]]
            A[np.arange(NLOC), cnt_loc] = c * NP + np.arange(NLOC)
            flat = np.full(KPAD * 128, ZROW, dtype=np.int64)
            flat[obase[-1]:] = -1  # trailing pad: trimmed by gather ucode
            for j in range(dmax):
                cj = int(c_j[j])
                if cj:
                    flat[obase[j]:obase[j] + cj] = A[order[:cj], j]
            for w in range(NW):
                wl = flat[w * WSLOTS * 128:(w + 1) * WSLOTS * 128]
                idxs[c, t, w] = np.tile(
                    wl.reshape(WSLOTS * 8, 16).T, (8, 1)).astype(np.int16)
            dn = np.ones(NP, dtype=np.float32)
            dn[:NLOC] = dloc
            deg_node[t, :, c * SL:(c + 1) * SL] = dn.reshape(SL, 128).T
            dr = np.ones(NP, dtype=np.float32)
            dr[:NLOC] = dloc[order]
            deg_rank[c, t] = dr.reshape(SL, 128).T
            rn = np.arange(NP, dtype=np.int64)
            rn[:NLOC] = order
            rank_node[c, t] = np.tile(
                rn.reshape(NP // 16, 16).T, (8, 1)).astype(np.int16)

    # feature-major fp16 x, padded per-core to NP rows: xT[t, f, row]
    xT = np.zeros((T, F, G), dtype=np.float16)
    xtr = x.transpose(0, 2, 1).astype(np.float16)  # [T, F, N]
    for c in range(NCORES):
        xT[:, :, c * NP:c * NP + NLOC] = xtr[:, :, c * NLOC:(c + 1) * NLOC]

    common = {
        "xT": xT,
        "deg_node": deg_node,
        "w_gcn": np.ascontiguousarray(np.asarray(W_gcn), dtype=np.float32),
        "w_ihT": np.ascontiguousarray(np.asarray(W_ih).T, dtype=np.float32),
        "w_hhT": np.ascontiguousarray(np.asarray(W_hh).T, dtype=np.float32),
        "b_ih": np.asarray(b_ih, dtype=np.float32).reshape(-1),
        "b_hh": np.asarray(b_hh, dtype=np.float32).reshape(-1),
        "b_gcn": np.asarray(b_gcn, dtype=np.float32).reshape(-1),
    }
    return [dict(common, idxs=idxs[c], deg_rank=deg_rank[c],
                 rank_node=rank_node[c]) for c in range(NCORES)]


# ------------------------------------------------------------- builder

def _build(reps=1):
    SH = G // 128                     # 160 node blocks
    NAC = SH // ACHUNK                # 20 Stage-A chunks
    NCH = math.ceil(NP / LSTM_CHUNK)
    sbase = np.concatenate([[0], np.cumsum(SLOTS)])
    wbounds = [min(K, i * WSLOTS) for i in range(NW + 1)]
    win_adds = [[] for _ in range(NW)]
    for j in range(len(SLOTS)):
        s0, s1 = int(sbase[j]), int(sbase[j + 1])
        for w in range(NW):
            a, b = max(s0, wbounds[w]), min(s1, wbounds[w + 1])
            if a < b:
                win_adds[w].append((a - wbounds[w], b - wbounds[w], a - s0, j))

    nc = bacc.Bacc("TRN2", target_bir_lowering=False, debug=False,
                   num_devices=NCORES, num_swdge_queues=NQ)
    xt_ext = nc.dram_tensor("xT", [T, F, G], FP16, kind="ExternalInput").ap()
    degn_ext = nc.dram_tensor("deg_node", [T, 128, SH], FP32,
                              kind="ExternalInput").ap()
    idx_ext = nc.dram_tensor("idxs", [T, NW, 128, WSLOTS * 8], I16,
                             kind="ExternalInput").ap()
    degr_ext = nc.dram_tensor("deg_rank", [T, 128, SL], FP32,
                              kind="ExternalInput").ap()
    rkn_ext = nc.dram_tensor("rank_node", [T, 128, NP // 16], I16,
                             kind="ExternalInput").ap()
    wg_ext = nc.dram_tensor("w_gcn", [F, H], FP32, kind="ExternalInput").ap()
    wih_ext = nc.dram_tensor("w_ihT", [H, G4], FP32, kind="ExternalInput").ap()
    whh_ext = nc.dram_tensor("w_hhT", [H, G4], FP32, kind="ExternalInput").ap()
    bih_ext = nc.dram_tensor("b_ih", [G4], FP32, kind="ExternalInput").ap()
    bhh_ext = nc.dram_tensor("b_hh", [G4], FP32, kind="ExternalInput").ap()
    bg_ext = nc.dram_tensor("b_gcn", [H], FP32, kind="ExternalInput").ap()
    ys_ext = nc.dram_tensor("ys", [T, H, NP], FP16, kind="ExternalOutput").ap()

    hfull = [nc.dram_tensor(f"hfull{t}", [G + 1, H], FP32).ap()
             for t in range(T)]
    gcnb = [nc.dram_tensor(f"gcnb{t}", [NP, H], FP32).ap() for t in range(T)]

    with tile.TileContext(nc) as tc:
        with tc.tile_pool(name="const", bufs=1) as const, \
             tc.tile_pool(name="xtp", bufs=3) as xtp, \
             tc.tile_pool(name="hp", bufs=3) as hp, \
             tc.tile_pool(name="idxp", bufs=4) as idxp, \
             tc.tile_pool(name="slabp", bufs=3) as slabp, \
             tc.tile_pool(name="accp", bufs=2) as accp, \
             tc.tile_pool(name="gcnp", bufs=2) as gcnp, \
             tc.tile_pool(name="up", bufs=2) as up, \
             tc.tile_pool(name="dvp", bufs=2) as dvp, \
             tc.tile_pool(name="smallp", bufs=2) as smallp, \
             tc.tile_pool(name="ps_a", bufs=2, space="PSUM") as ps_a, \
             tc.tile_pool(name="ps_tr", bufs=2, space="PSUM") as ps_tr, \
             tc.tile_pool(name="ps_g", bufs=2, space="PSUM") as ps_g:

            nc.gpsimd.load_library(mlp_lib)
            ident32 = const.tile([128, 128], FP32)
            make_identity(nc, ident32[:])
            wg_sb = const.tile([F, H], FP16)
            nc.gpsimd.dma_start(out=wg_sb[:], in_=wg_ext[:])
            wih_sb = const.tile([H, G4], FP16)
            nc.gpsimd.dma_start(out=wih_sb[:], in_=wih_ext[:])
            whh_sb = const.tile([H, G4], FP16)
            nc.gpsimd.dma_start(out=whh_sb[:], in_=whh_ext[:])
            bsl = G4 // 128
            bih_sb = const.tile([128, bsl], FP32)
            nc.sync.dma_start(out=bih_sb[:],
                              in_=bih_ext.rearrange("(s p) -> p s", p=128))
            bhh_sb = const.tile([128, bsl], FP32)
            nc.sync.dma_start(out=bhh_sb[:],
                              in_=bhh_ext.rearrange("(s p) -> p s", p=128))
            badd = const.tile([128, bsl], FP32)
            nc.vector.tensor_add(out=badd[:], in0=bih_sb[:], in1=bhh_sb[:])
            bg_row = const.tile([1, H], FP32)
            nc.sync.dma_start(out=bg_row[:], in_=bg_ext[None, :])
            bg_sb = const.tile([128, H], FP32)
            nc.gpsimd.partition_broadcast(out_ap=bg_sb[:], in_ap=bg_row[:])
            zrow = const.tile([1, H], FP32)
            nc.vector.memset(zrow[:], 0.0)
            zblk = const.tile([128, SL, H], FP32)
            nc.vector.memset(zblk[:], 0.0)
            for t in range(T):
                nc.sync.dma_start(out=hfull[t][G:G + 1, :], in_=zrow[:])

            c_sb = const.tile([H, NP], FP32, tag="c_state")
            h16 = const.tile([H, NP], FP16, tag="h_state")

            def stage_a(t):
                """h' = dinv * (xT.T @ W_gcn) -> hfull[t], feature-major in."""
                degn = smallp.tile([128, SH], FP32, tag="degn")
                nc.sync.dma_start(out=degn[:], in_=degn_ext[t])
                sq_n = smallp.tile([128, SH], FP32, tag="sqn")
                nc.scalar.activation(out=sq_n[:], in_=degn[:], func=AF.Sqrt)
                dinv_n = smallp.tile([128, SH], FP32, tag="dinvn")
                nc.vector.reciprocal(out=dinv_n[:], in_=sq_n[:])
                for ac in range(NAC):
                    s0 = ac * ACHUNK
                    s1 = s0 + ACHUNK
                    xts = xtp.tile([F, ACHUNK * 128], FP16, tag="xts")
                    nc.sync.dma_start(out=xts[:],
                                      in_=xt_ext[t, :, s0 * 128:s1 * 128])
                    h_ps = ps_a.tile([128, ACHUNK, H], FP32, space="PSUM",
                                     tag="psa")
                    for s in range(s0, s1):
                        nc.tensor.matmul(
                            out=h_ps[:, s - s0, :],
                            lhsT=xts[:, (s - s0) * 128:(s - s0 + 1) * 128],
                            rhs=wg_sb[:], start=True, stop=True)
                    hl = hp.tile([128, ACHUNK, H], FP32, tag="hl")
                    nc.vector.tensor_tensor(
                        out=hl[:], in0=h_ps[:],
                        in1=dinv_n[:, s0:s1, None].to_broadcast(
                            [128, ACHUNK, H]),
                        op=OP.mult)
                    nc.sync.dma_start(
                        out=hfull[t][s0 * 128:s1 * 128, :]
                        .rearrange("(s p) h -> p s h", p=128),
                        in_=hl[:])

            def stage_b(t):
                """Slab gathers + DVE accumulation (rank-major)."""
                acc = accp.tile([128, SL, H], FP32, tag="acc")
                for w in range(NW):
                    idx_sb = idxp.tile([128, WSLOTS * 8], I16, tag="idx")
                    nc.sync.dma_start(out=idx_sb[:], in_=idx_ext[t, w])
                    slab = slabp.tile([128, WSLOTS, H], FP32, tag="slab")
                    valid_w = max(0, min(WSLOTS * 128,
                                         K * 128 - w * WSLOTS * 128))
                    nc.gpsimd.dma_gather(slab[:], hfull[t][:, :], idx_sb[:],
                                         WSLOTS * 128, valid_w, H,
                                         single_packet=False,
                                         queue_num=w % NQ)
                    for (a, b, accs, j) in win_adds[w]:
                        ln = b - a
                        if j == 0:
                            nc.vector.tensor_copy(
                                out=acc[:, accs:accs + ln, :],
                                in_=slab[:, a:b, :])
                        else:
                            nc.vector.tensor_add(
                                out=acc[:, accs:accs + ln, :],
                                in0=acc[:, accs:accs + ln, :],
                                in1=slab[:, a:b, :])
                return acc

            def stage_cd(t, acc):
                """Scale/bias/relu, unpermute to node order, LSTM step."""
                degr = smallp.tile([128, SL], FP32, tag="degr")
                nc.sync.dma_start(out=degr[:], in_=degr_ext[t])
                sq_r = smallp.tile([128, SL], FP32, tag="sqr")
                nc.scalar.activation(out=sq_r[:], in_=degr[:], func=AF.Sqrt)
                dinv_r = smallp.tile([128, SL], FP32, tag="dinvr")
                nc.vector.reciprocal(out=dinv_r[:], in_=sq_r[:])
                nc.vector.tensor_tensor(
                    out=acc[:], in0=acc[:],
                    in1=dinv_r[:, :, None].to_broadcast([128, SL, H]),
                    op=OP.mult)
                nc.vector.tensor_tensor(
                    out=acc[:], in0=acc[:],
                    in1=bg_sb[:, None, :].to_broadcast([128, SL, H]),
                    op=OP.add)
                gcn_r = gcnp.tile([128, SL, H], FP32, tag="gcnr")
                nc.scalar.activation(out=gcn_r[:], in_=acc[:], func=AF.Relu)
                rkn_sb = smallp.tile([128, NP // 16], I16, tag="rkn")
                nc.sync.dma_start(out=rkn_sb[:], in_=rkn_ext[t])
                nc.sync.dma_start(
                    out=gcnb[t][:, :].rearrange("(s p) h -> p s h", p=128),
                    in_=zblk[:])
                nc.gpsimd.dma_scatter_add(
                    gcnb[t][:, :], gcn_r[:], rkn_sb[:], NP, NP, H)
                gcn_nm = gcnp.tile([128, SL, H], FP32, tag="gcnnm")
                nc.sync.dma_start(
                    out=gcn_nm[:],
                    in_=gcnb[t][:, :].rearrange("(s p) h -> p s h", p=128))
                uT = up.tile([H, NP], FP16, tag="uT")
                for s in range(SL):
                    u_ps = ps_tr.tile([128, 128], FP32, space="PSUM",
                                      tag="tr32")
                    nc.tensor.transpose(out=u_ps[0:H, :], in_=gcn_nm[:, s, :],
                                        identity=ident32[:])
                    nc.scalar.activation(out=uT[:, s * 128:(s + 1) * 128],
                                         in_=u_ps[0:H, :], func=AF.Copy)

                # LSTM step (PyTorch gate order i,f,g,o; badd = b_ih + b_hh)
                for chi in range(NCH):
                    c0 = chi * LSTM_CHUNK
                    c1 = min(NP, c0 + LSTM_CHUNK)
                    w = c1 - c0
                    ps_if = ps_g.tile([128, LSTM_CHUNK], FP32, space="PSUM",
                                      tag="psif")
                    nc.tensor.matmul(out=ps_if[:, :w], lhsT=wih_sb[:, 0:128],
                                     rhs=uT[:, c0:c1], start=True, stop=False)
                    nc.tensor.matmul(out=ps_if[:, :w], lhsT=whh_sb[:, 0:128],
                                     rhs=h16[:, c0:c1], start=False, stop=True)
                    ps_go = ps_g.tile([128, LSTM_CHUNK], FP32, space="PSUM",
                                      tag="psgo")
                    nc.tensor.matmul(out=ps_go[:, :w], lhsT=wih_sb[:, 128:G4],
                                     rhs=uT[:, c0:c1], start=True, stop=False)
                    nc.tensor.matmul(out=ps_go[:, :w], lhsT=whh_sb[:, 128:G4],
                                     rhs=h16[:, c0:c1], start=False, stop=True)
                    sig_i = dvp.tile([H, LSTM_CHUNK], FP32, tag="sigi")
                    nc.scalar.activation(out=sig_i[:, :w], in_=ps_if[0:H, :w],
                                         func=AF.Sigmoid, bias=badd[0:H, 0:1])
                    sig_f = dvp.tile([H, LSTM_CHUNK], FP32, tag="sigf")
                    nc.scalar.activation(out=sig_f[:, :w], in_=ps_if[H:128, :w],
                                         func=AF.Sigmoid, bias=badd[H:128, 0:1])
                    tanh_g = dvp.tile([H, LSTM_CHUNK], FP32, tag="tanhg")
                    nc.scalar.activation(out=tanh_g[:, :w], in_=ps_go[0:H, :w],
                                         func=AF.Tanh, bias=badd[0:H, 1:2])
                    sig_o = dvp.tile([H, LSTM_CHUNK], FP32, tag="sigo")
                    nc.scalar.activation(out=sig_o[:, :w], in_=ps_go[H:128, :w],
                                         func=AF.Sigmoid, bias=badd[H:128, 1:2])
                    tmp1 = dvp.tile([H, LSTM_CHUNK], FP32, tag="tmp1")
                    nc.vector.tensor_mul(out=tmp1[:, :w], in0=sig_f[:, :w],
                                         in1=c_sb[:, c0:c1])
                    tmp2 = dvp.tile([H, LSTM_CHUNK], FP32, tag="tmp2")
                    nc.vector.tensor_mul(out=tmp2[:, :w], in0=sig_i[:, :w],
                                         in1=tanh_g[:, :w])
                    nc.vector.tensor_add(out=c_sb[:, c0:c1], in0=tmp1[:, :w],
                                         in1=tmp2[:, :w])
                    tanh_c = dvp.tile([H, LSTM_CHUNK], FP32, tag="tanhc")
                    nc.scalar.activation(out=tanh_c[:, :w], in_=c_sb[:, c0:c1],
                                         func=AF.Tanh)
                    nc.vector.tensor_mul(out=h16[:, c0:c1], in0=sig_o[:, :w],
                                         in1=tanh_c[:, :w])
                nc.sync.dma_start(out=ys_ext[t], in_=h16[:])

            for rep in range(reps):
                for t in range(T):
                    stage_a(t)
                accs = {}
                for t in range(T):
                    if t == 0:
                        nc.vector.memset(c_sb[:], 0.0)
                        nc.vector.memset(h16[:], 0.0)
                    accs[t] = stage_b(t)
                    if t >= 1:
                        stage_cd(t - 1, accs.pop(t - 1))
                stage_cd(T - 1, accs.pop(T - 1))

    nc.compile()
    return nc


_NC_CACHE = {}


def kernel(x, edge_index, W_gcn, b_gcn, W_ih, W_hh, b_ih, b_hh, reps=1):
    in_maps = _host_prep(x, edge_index, W_gcn, b_gcn, W_ih, W_hh, b_ih, b_hh)
    if reps not in _NC_CACHE:
        _NC_CACHE[reps] = _build(reps)
    nc = _NC_CACHE[reps]
    res = run_bass_kernel_spmd(nc, in_maps, core_ids=list(range(NCORES)))
    out = np.concatenate(
        [res.results[c]["ys"][:, :, :NLOC].transpose(0, 2, 1)
         for c in range(NCORES)], axis=1)
    return out.astype(np.float32)
